# revision 2
# baseline (speedup 1.0000x reference)
"""Trainium2 Bass kernel for nn_Coo2Cel (periodic pairwise displacement grid).

Computes, for B=1, N=1024 atoms in a periodic box (diagonal cell, 27 lattice
shifts): out[b,i,j,s,:] = (vec, sod) where vec = pos_i - pos_j - shift_s,
sod = |vec|^2, masked to zero unless 0 <= sod < rc^2 (with self-pair at the
zero shift excluded; that pair has vec == 0 exactly so masking is a no-op).

Sharding: query rows i are split row-wise across 8 NeuronCores (128 rows per
core, mapped to the 128 SBUF partitions). Every core holds all N candidate
positions (~12 KB). No cross-device communication.

Key structure exploited on-device: with a diagonal cell, the shift along each
axis is sigma_c * box_c with sigma_c in {-1,0,+1}, so
    vec[i,j,s,c]   = w[c, sigma_c(s)][i,j],   w = d_c - sigma*box_c
    sod[i,j,s]     = sq[x,sx] + sq[y,sy] + sq[z,sz],  sq = w*w
i.e. the entire N x N x 27 grid is built from 9 per-component [128, N] planes.
The mask (sod < rc^2) is fused into the output assembly with
scalar_tensor_tensor: out = (sod is_lt rc^2) mult w — no mask tensor is ever
materialized.
"""
import sys

if "/opt/trn_rl_repo" not in sys.path:
    sys.path.insert(0, "/opt/trn_rl_repo")

import numpy as np

N = 1024          # atoms
S = 27            # lattice shifts
P = 128           # partitions / query rows per core
NCORES = 8
RC2 = 36.0        # rc^2, rc = 6.0
JT = 64           # candidate tile size (16 tiles / 16 output DMAs)
NT = N // JT

TRACE = False          # set by test harness to collect a profile
LAST_RESULT = None     # BassKernelResults of the last run (for profiling)

_CACHE = {}


def _build(box, pbc_tuple, jts=None, outt_bufs=3, work_bufs=2, repeat=1):
    jts = (jts or [JT] * NT) * repeat
    import concourse.bacc as bacc
    import concourse.mybir as mybir
    from concourse.tile import TileContext

    F32 = mybir.dt.float32
    ADD = mybir.AluOpType.add
    MULT = mybir.AluOpType.mult
    SUB = mybir.AluOpType.subtract
    ISLT = mybir.AluOpType.is_lt

    nc = bacc.Bacc()
    # single input: candidate planes [c, 0:N] plus this core's query column
    # at [c, N] (one DMA -> one semaphore wait on the first compute op)
    pin_d = nc.declare_dram_parameter("pin", [P, 3, N + 1], F32, isOutput=False)
    out_d = nc.declare_dram_parameter("out", [P, N * S * 4], F32, isOutput=True)

    with TileContext(nc) as tc:
        with (
            tc.tile_pool(name="const", bufs=1) as cpool,
            tc.tile_pool(name="work", bufs=work_bufs) as wpool,
            tc.tile_pool(name="outp", bufs=outt_bufs) as opool,
        ):
            pin = cpool.tile([P, 3, N + 1], F32)
            w = cpool.tile([P, 3, 3, N], F32)   # [c, sigma', j], sigma' = sigma+1
            # input DMA on the SWDGE (gpsimd) path: keeps all 8 HWDGE
            # completion-semaphore lanes free for the 8 output DMAs
            nc.gpsimd.dma_start(out=pin[:], in_=pin_d[:])

            def build_w(jsl, n):
                for c in range(3):
                    # d_c = q_c - p_c (broadcast query column along j)
                    nc.vector.tensor_tensor(
                        out=w[:, c, 1, jsl],
                        in0=pin[:, c, N:N + 1].broadcast_to([P, n]),
                        in1=pin[:, c, jsl],
                        op=SUB,
                    )
                    # w[c,0] = d + box (sigma=-1), w[c,2] = d - box (sigma=+1)
                    nc.vector.tensor_scalar_add(
                        out=w[:, c, 0, jsl], in0=w[:, c, 1, jsl],
                        scalar1=float(box[c]))
                    nc.vector.tensor_scalar_sub(
                        out=w[:, c, 2, jsl], in0=w[:, c, 1, jsl],
                        scalar1=float(box[c]))

            # build tile 0's slice of w first so the first output DMA can
            # launch without waiting on the full-width plane setup (ramp)
            build_w(slice(0, jts[0]), jts[0])
            build_w(slice(jts[0], N), N - jts[0])
            j0 = 0
            for jt, jts_n in enumerate(jts):
                j0 = j0 % N          # repeat>1 re-covers the same range
                js = slice(j0, j0 + jts_n)
                j0 += jts_n
                sq = wpool.tile([P, 3, 3, jts_n], F32, tag="sq")   # w^2 slices
                t = wpool.tile([P, 3, 3, jts_n], F32, tag="t")     # sqx+sqy
                sod = wpool.tile([P, S, jts_n], F32, tag="sod")    # [s, j]
                outt = opool.tile([P, jts_n, S, 4], F32, tag="outt")
                # First-writer memset: absorbs the DMA slot-recycling wait on
                # an instruction with no data inputs (TT/STT ISA structs have
                # a single sync-wait slot).
                nc.vector.memset(outt[:, 0, 0, 0:1], 0.0)
                # squares on ScalarE (own SBUF port; DVE is the bottleneck)
                nc.scalar.activation(
                    out=sq[:], in_=w[:, :, :, js],
                    func=mybir.ActivationFunctionType.Square)
                # non-periodic axes: make every shifted image fail the rc cut
                for c in range(3):
                    if not pbc_tuple[c]:
                        for sig in (0, 2):
                            nc.vector.tensor_scalar_add(
                                out=sq[:, c, sig, :], in0=sq[:, c, sig, :],
                                scalar1=1e9)
                for sx in range(3):
                    # t[sx, sy, j] = sq_x_sx[j] + sq_y_sy[j]
                    nc.vector.tensor_tensor(
                        out=t[:, sx],
                        in0=sq[:, 0, sx, :].unsqueeze(1).broadcast_to([P, 3, jts_n]),
                        in1=sq[:, 1, :, :],
                        op=ADD,
                    )
                    # sod[9sx+3sy+sz, j] = t[sx, sy, j] + sq_z_sz[j]
                    nc.vector.tensor_tensor(
                        out=sod[:, 9 * sx:9 * (sx + 1), :].rearrange(
                            "p (sy sz) j -> p sy sz j", sz=3),
                        in0=t[:, sx].unsqueeze(2).broadcast_to([P, 3, 3, jts_n]),
                        in1=sq[:, 2, :, :].unsqueeze(1).broadcast_to([P, 3, 3, jts_n]),
                        op=ADD,
                    )
                # masked assembly: out = (sod < rc^2) * w, interleaved [j, s, c]
                # (STT is limited to <=2 free dims, so group the 27 shifts
                #  into uniform-stride runs per component)
                def stt(o, i0, i1):
                    nc.vector.scalar_tensor_tensor(
                        out=o, in0=i0, scalar=RC2, in1=i1,
                        op0=ISLT, op1=MULT)
                for sig in range(3):
                    # c=0: sigma_x = sig -> s in [9*sig, 9*sig+9), contiguous
                    stt(
                        outt[:, :, 9 * sig:9 * (sig + 1), 0].transpose([0, 2, 1]),
                        sod[:, 9 * sig:9 * (sig + 1), :],
                        w[:, 0, sig, js].unsqueeze(1).broadcast_to([P, 9, jts_n]),
                    )
                    # c=2: sigma_z = sig -> s = sig (mod 3), stride 3
                    stt(
                        outt[:, :, sig:S:3, 2].transpose([0, 2, 1]),
                        sod[:, sig:S:3, :],
                        w[:, 2, sig, js].unsqueeze(1).broadcast_to([P, 9, jts_n]),
                    )
                    # c=1: sigma_y = sig -> s in 9*sx + 3*sig + {0,1,2}
                    for sx in range(3):
                        s0 = 9 * sx + 3 * sig
                        stt(
                            outt[:, :, s0:s0 + 3, 1].transpose([0, 2, 1]),
                            sod[:, s0:s0 + 3, :],
                            w[:, 1, sig, js].unsqueeze(1).broadcast_to([P, 3, jts_n]),
                        )
                stt(
                    outt[:, :, :, 3].transpose([0, 2, 1]),
                    sod[:], sod[:],
                )
                nc.sync.dma_start(
                    out=out_d[:, js.start * S * 4:js.stop * S * 4],
                    in_=outt[:].rearrange("p j s c -> p (j s c)"),
                )
    nc.finalize()
    return nc


def _in_maps(pos_cel, cel_mat):
    # Cartesian positions (fp32). For the diagonal cell this is exact:
    # pos[:, d] = pos_cel[:, d] * box_d (+ exact zeros).
    pos = pos_cel[0].astype(np.float32) @ cel_mat[0]
    pos = pos.astype(np.float32)
    in_maps = []
    for k in range(NCORES):
        pin = np.empty((P, 3, N + 1), dtype=np.float32)
        pin[:, :, :N] = pos.T[None]
        pin[:, :, N] = pos[k * P:(k + 1) * P]
        in_maps.append({"pin": pin})
    return in_maps


def kernel(pos_cel, cel_mat, pbc):
    global LAST_RESULT
    from concourse.bass_utils import run_bass_kernel_spmd

    pos_cel = np.asarray(pos_cel)
    cel_mat = np.asarray(cel_mat, dtype=np.float32)
    pbc = np.asarray(pbc)
    B = pos_cel.shape[0]
    assert pos_cel.shape == (B, N, 3), pos_cel.shape
    assert B == 1

    off = cel_mat[0] - np.diag(np.diag(cel_mat[0]))
    assert np.all(off == 0), "kernel assumes a diagonal cell matrix"
    box = tuple(float(cel_mat[0][c, c]) for c in range(3))
    pbc_tuple = tuple(bool(x) for x in pbc[0])

    key = (box, pbc_tuple)
    if key not in _CACHE:
        _CACHE[key] = _build(box, pbc_tuple)
    nc = _CACHE[key]

    in_maps = _in_maps(pos_cel, cel_mat)

    res = run_bass_kernel_spmd(nc, in_maps, list(range(NCORES)), trace=TRACE)
    LAST_RESULT = res

    out = np.empty((1, N, N, S, 4), dtype=np.float32)
    for k in range(NCORES):
        out[0, k * P:(k + 1) * P] = np.asarray(
            res.results[k]["out"]).reshape(P, N, S, 4)
    return out

